# revision 3
# baseline (speedup 1.0000x reference)
"""Trainium2 Bass kernel for nn_CellLayer_25752623907073.

The reference is an init-guess network (MLP/S4D stack) followed by a DEER
quasi-Newton parallel solve of a GRU recurrence.  DEER is a contraction: it
converges to the sequential GRU trajectory from any initial guess, so the
init-guess network has no effect on the output and the task reduces to
evaluating the GRU trajectory.

The kernel solves the GRU by quasi-DEER fixed-point iteration with a
DIAGONAL linear solve: each round evaluates all gates in parallel at the
lagged previous iterate h~[t-1], then propagates the exact diagonal
recurrence h[t] = z[t]*h[t-1] + (1-z[t])*a[t] along the sequence with DVE
tensor_tensor_scan ops (state fp32 inside the scan).  The fixed point is
the true trajectory; measured contraction ~0.26/round, so ROUNDS=4 puts
iteration error under the bf16 floor (total rel err ~8e-3 vs 2e-2 gate).

Sharding: 8 cores = 4 batches x 2 sequence halves, no collectives.  Each
core owns a window of MARG+1024 positions (MARG=16 warm-in cols, discarded:
zero-padded+masked on first-half cores, real inputs on second-half cores
whose initial-state error decays through them).  The window is split into
two 528-col segments stacked on partitions (seg0 -> 0:64, seg1 -> 64:128),
so each ACT/DVE op covers 2x columns per instruction; matmuls use
block-diagonal stationaries.  Segment scan initials are 0 every round: the
error this injects decays below 1e-3 within the MARG warm-in cols.

Tiles are asymmetric [132, 396]: the round-to-round critical cycle runs
through tile 0 only (its scan feeds the next round's h-matmuls), so tile 0
is small to keep the chain short while tile 1 rides the engine slack.
Emission order per engine follows steady-state readiness (the completion-
counter semaphores entangle waiters when instructions park out of order).
The last round's tile-1 scan is split into 132-col chunks so the output
DMAs (bf16, converted on host) pipeline behind compute on the SP/ACT/Pool
HWDGE queues.
"""

import numpy as np
import ml_dtypes

import concourse.bacc as bacc
import concourse.mybir as mybir
import concourse.tile as tile
from concourse.bass_utils import run_bass_kernel_spmd

F32 = mybir.dt.float32
BF16 = mybir.dt.bfloat16
AF = mybir.ActivationFunctionType
ALU = mybir.AluOpType

B, L, NIN, H = 4, 2048, 32, 64
TPC = L // 2          # timesteps per core
MARG = 16             # warm-in columns per segment (discarded)
SL = MARG + 512       # segment length (528)
NW = MARG + TPC       # window length (1040)
NSEG = 2
TLS = [132, 396]      # tile column sizes (sum = SL)
FIN_CHUNK = 132       # last-round scan/DMA chunk size
ROUNDS = 4
N_CORES = 8
XR = 2 * (NIN + 1)    # x2 rows: 2 segs x (x + ones)

# blob columns (bf16, [128, BLOBCOLS]) — DMA'd in 4 pieces:
#   A: [0:514]  wxr|wxz|wxa block-diag (rows: Wg^T + bias row, per seg),
#               I128, bn2, flag           (Pool queue, needed by round 0)
#   B: x2 tile-0 cols                      (SP queue)
#   C: x2 tile-1 cols                      (SP queue)
#   D: uhr|uhz|uha block-diag              (ACT queue, needed by round 1)
WOFF = {"wxr": 0, "wxz": 128, "wxa": 256, "ident": 384}
BNOFF = 512
FLAGOFF = 513
XOFF = 514
UOFF = {"uhr": XOFF + SL, "uhz": XOFF + SL + 128, "uha": XOFF + SL + 256}
BLOBCOLS = XOFF + SL + 384


def _build_program():
    nc = bacc.Bacc("TRN2", debug=False)

    wx = nc.declare_dram_parameter("wx", [128, BLOBCOLS], BF16, isOutput=False)
    yout = nc.declare_dram_parameter("y", [H, L // 2], BF16, isOutput=True)

    with tile.TileContext(nc) as tc:
        with (
            tc.tile_pool(name="const", bufs=1) as cpool,
            tc.tile_pool(name="tmp", bufs=2) as tmp,
            tc.tile_pool(name="psum_r", bufs=2, space="PSUM") as psum_r,
            tc.tile_pool(name="psum_z", bufs=2, space="PSUM") as psum_z,
            tc.tile_pool(name="psum_a1", bufs=2, space="PSUM") as psum_a1,
            tc.tile_pool(name="psum_ia", bufs=2, space="PSUM") as psum_ia,
        ):
            t_wx = cpool.tile([128, BLOBCOLS], BF16)
            nc.gpsimd.dma_start(t_wx[:, 0:XOFF], wx[:, 0:XOFF])
            nc.sync.dma_start(t_wx[:, XOFF:XOFF + TLS[0]],
                              wx[:, XOFF:XOFF + TLS[0]])
            nc.sync.dma_start(t_wx[:, XOFF + TLS[0]:XOFF + SL],
                              wx[:, XOFF + TLS[0]:XOFF + SL])
            nc.scalar.dma_start(t_wx[:, XOFF + SL:BLOBCOLS],
                                wx[:, XOFF + SL:BLOBCOLS])

            wst = {k: t_wx[:, off:off + 128] for k, off in UOFF.items()}
            wst["ident"] = t_wx[:, WOFF["ident"]:WOFF["ident"] + 128]
            wx_x = {k: t_wx[0:XR, WOFF[k]:WOFF[k] + 128]
                    for k in ("wxr", "wxz", "wxa")}

            # PE p-state warm-up: dummy matmuls on a zeroed tile so the ramp
            # to full clock runs during the input DMA.
            t_zero = cpool.tile([128, 264], BF16)
            nc.vector.memset(t_zero[:], 0.0)
            for i in range(2):
                p_w = psum_ia.tile([128, 264], F32, tag="pia", name=f"warm{i}")
                nc.tensor.matmul(p_w[:], t_zero[:, 0:128], t_zero[:],
                                 start=True, stop=True)

            # warm the sigmoid/tanh ACT tables during the input DMA
            t_warm = cpool.tile([1, 2], F32)
            nc.vector.memset(t_warm[:], 0.0)
            nc.scalar.activation(t_warm[:, 0:1], t_warm[:, 0:1], AF.Sigmoid)
            nc.scalar.activation(t_warm[:, 1:2], t_warm[:, 1:2], AF.Tanh)

            # bn/flag as f32 via DVE so sweep ops never wait on the DMA sem
            t_bnflag = cpool.tile([128, 2], F32)
            nc.vector.tensor_copy(t_bnflag[:], t_wx[:, BNOFF:BNOFF + 2])
            t_bn = t_bnflag[:, 0:1]
            t_flag = t_bnflag[0:H, 1:2]

            # h~ double buffers: col 0 = 0 (state before the segment), col
            # 1+c = state at segment-local col c.
            hbuf = [cpool.tile([128, 1 + SL], BF16, name=f"h{i}")
                    for i in range(2)]
            nc.vector.memset(hbuf[0][:, 0:1], 0.0)
            nc.vector.memset(hbuf[1][:, 0:1], 0.0)

            def x2(c0, tl):
                return t_wx[0:XR, XOFF + c0:XOFF + c0 + tl]

            def emit_round(k):
                hprev = hbuf[k % 2]
                hnew = hbuf[(k + 1) % 2]
                ntile = len(TLS)
                ps = {}
                gate = {}
                # PE: all tiles' pre-act matmuls in readiness order, the
                # ident-adds (which wait on t1) after them.
                c0 = 0
                for t, tl in enumerate(TLS):
                    hp = hprev[:, c0:c0 + tl]
                    xa = x2(c0, tl)
                    p_r = psum_r.tile([128, tl], F32, tag="pr",
                                      name=f"pr_{k}_{t}")
                    p_z = psum_z.tile([128, tl], F32, tag="pz",
                                      name=f"pz_{k}_{t}")
                    p_ia = psum_ia.tile([128, tl], F32, tag="pia",
                                        name=f"pia_{k}_{t}")
                    p_a1 = None
                    if k > 0:
                        p_a1 = psum_a1.tile([128, tl], F32, tag="pa1",
                                            name=f"pa1_{k}_{t}")
                        nc.tensor.matmul(p_r[:], wst["uhr"], hp,
                                         start=True, stop=False)
                        nc.tensor.matmul(p_r[:], wx_x["wxr"], xa,
                                         start=False, stop=True)
                        nc.tensor.matmul(p_a1[:], wst["uha"], hp,
                                         start=True, stop=True,
                                         skip_group_check=True)
                        nc.tensor.matmul(p_z[:], wst["uhz"], hp,
                                         start=True, stop=False,
                                         skip_group_check=True)
                        nc.tensor.matmul(p_z[:], wx_x["wxz"], xa,
                                         start=False, stop=True,
                                         skip_group_check=True)
                    else:
                        nc.tensor.matmul(p_r[:], wx_x["wxr"], xa,
                                         start=True, stop=True)
                        nc.tensor.matmul(p_z[:], wx_x["wxz"], xa,
                                         start=True, stop=True,
                                         skip_group_check=True)
                    nc.tensor.matmul(p_ia[:], wx_x["wxa"], xa,
                                     start=True, stop=False,
                                     skip_group_check=True)
                    ps[t] = (p_r, p_z, p_a1, p_ia)
                    c0 += tl

                # ACT sigmoids for r: tile order; z(t0); then tanh(t0)
                # before z(t1)/tanh(t1) (readiness order).
                for t in range(ntile):
                    p_r, p_z, p_a1, p_ia = ps[t]
                    tl = TLS[t]
                    rt = tmp.tile([128, tl], BF16, tag="rt", name=f"rt{k}_{t}")
                    zt = tmp.tile([128, tl], BF16, tag="zt", name=f"zt{k}_{t}")
                    t1 = tmp.tile([128, tl], BF16, tag="t1", name=f"t1{k}_{t}")
                    at = tmp.tile([128, tl], BF16, tag="at", name=f"at{k}_{t}")
                    gt = tmp.tile([128, tl], BF16, tag="gt", name=f"gt{k}_{t}")
                    gate[t] = (rt, zt, t1, at, gt)

                nc.scalar.activation(gate[0][0][:], ps[0][0][:], AF.Sigmoid)
                nc.scalar.activation(gate[0][1][:], ps[0][1][:], AF.Sigmoid)

                # DVE t1 ops interleave with ACT by readiness
                def emit_t1(t):
                    rt, zt, t1, at, gt = gate[t]
                    p_r, p_z, p_a1, p_ia = ps[t]
                    if k > 0:
                        nc.vector.scalar_tensor_tensor(
                            t1[:], in0=p_a1[:], scalar=t_bn, in1=rt[:],
                            op0=ALU.add, op1=ALU.mult)
                    else:
                        nc.vector.tensor_scalar_mul(t1[:], rt[:], t_bn)
                    nc.tensor.matmul(p_ia[:], wst["ident"], t1[:],
                                     start=False, stop=True,
                                     skip_group_check=True)

                emit_t1(0)
                nc.scalar.activation(gate[1][0][:], ps[1][0][:], AF.Sigmoid)
                nc.scalar.activation(gate[0][3][:], ps[0][3][:], AF.Tanh)
                emit_t1(1)
                nc.scalar.activation(gate[1][1][:], ps[1][1][:], AF.Sigmoid)
                nc.scalar.activation(gate[1][3][:], ps[1][3][:], AF.Tanh)

                # DVE: g/mask/scan per tile in readiness order; the last
                # round's tile-1 scan is chunked so output DMAs pipeline.
                dmaq = [nc.sync, nc.scalar, nc.gpsimd]
                dmai = [0]

                def emit_dma(blo, bhi):
                    ylo, yhi = blo - 1 - MARG, bhi - 1 - MARG
                    q0 = dmaq[dmai[0] % 3]
                    q1 = dmaq[(dmai[0] + 1) % 3]
                    dmai[0] += 2
                    q0.dma_start(yout[:, ylo:yhi], hnew[0:H, blo:bhi])
                    q1.dma_start(yout[:, 512 + ylo:512 + yhi],
                                 hnew[H:128, blo:bhi])

                rt0, zt0, t10, at0, gt0 = gate[0]
                nc.vector.scalar_tensor_tensor(
                    gt0[:], in0=zt0[:], scalar=1.0, in1=at0[:],
                    op0=ALU.subtract, op1=ALU.mult)
                nc.vector.tensor_scalar_mul(
                    gt0[0:H, 0:MARG], gt0[0:H, 0:MARG], t_flag)
                nc.vector.tensor_tensor_scan(
                    hnew[:, 1:1 + TLS[0]], zt0[:], gt0[:], 0.0,
                    ALU.mult, ALU.subtract)
                if k == ROUNDS - 1:
                    emit_dma(1 + MARG, 1 + TLS[0])

                rt1, zt1, t11, at1, gt1 = gate[1]
                nc.vector.scalar_tensor_tensor(
                    gt1[:], in0=zt1[:], scalar=1.0, in1=at1[:],
                    op0=ALU.subtract, op1=ALU.mult)
                if k < ROUNDS - 1:
                    nc.vector.tensor_tensor_scan(
                        hnew[:, 1 + TLS[0]:1 + SL], zt1[:], gt1[:],
                        hnew[:, TLS[0]:TLS[0] + 1], ALU.mult, ALU.subtract)
                else:
                    c0 = TLS[0]
                    while c0 < SL:
                        ce = min(c0 + FIN_CHUNK, SL)
                        nc.vector.tensor_tensor_scan(
                            hnew[:, 1 + c0:1 + ce],
                            zt1[:, c0 - TLS[0]:ce - TLS[0]],
                            gt1[:, c0 - TLS[0]:ce - TLS[0]],
                            hnew[:, c0:c0 + 1], ALU.mult, ALU.subtract)
                        emit_dma(1 + c0, 1 + ce)
                        c0 = ce

            for k in range(ROUNDS):
                emit_round(k)

    nc.compile()
    return nc


_CACHE = {}


def kernel(**inputs):
    xs = np.asarray(inputs["xs"], np.float32)
    w_ih = np.asarray(inputs["w_ih"], np.float32)
    w_hh = np.asarray(inputs["w_hh"], np.float32)
    b_gru = np.asarray(inputs["b_gru"], np.float32)
    bn_gru = np.asarray(inputs["bn_gru"], np.float32)

    if "nc" not in _CACHE:
        _CACHE["nc"] = _build_program()
    nc = _CACHE["nc"]

    base = np.zeros((128, BLOBCOLS), np.float32)
    for gi, key in enumerate(("wxr", "wxz", "wxa")):
        wg = w_ih[gi * H:(gi + 1) * H]          # (H, NIN)
        bg = b_gru[gi * H:(gi + 1) * H]
        for s in range(NSEG):
            r0 = s * (NIN + 1)
            cblk = WOFF[key] + s * H
            base[r0:r0 + NIN, cblk:cblk + H] = wg.T
            base[r0 + NIN, cblk:cblk + H] = bg
    for gi, key in enumerate(("uhr", "uhz", "uha")):
        ug = w_hh[gi * H:(gi + 1) * H]          # (H, H)
        for s in range(NSEG):
            base[s * H:(s + 1) * H,
                 UOFF[key] + s * H:UOFF[key] + (s + 1) * H] = ug.T
    base[:, WOFF["ident"]:WOFF["ident"] + 128] = np.eye(128, dtype=np.float32)
    base[0:H, BNOFF] = bn_gru
    base[H:128, BNOFF] = bn_gru

    in_maps = []
    for core in range(N_CORES):
        bi, half = core // 2, core % 2
        p0 = half * TPC - MARG
        xw = np.zeros((NW, NIN), np.float32)
        lo = max(0, p0)
        xw[lo - p0:] = xs[bi, lo:p0 + NW]
        blob = base.copy()
        for s in range(NSEG):
            r0 = s * (NIN + 1)
            xsg = xw[512 * s:512 * s + SL]       # (SL, NIN)
            blob[r0:r0 + NIN, XOFF:XOFF + SL] = xsg.T
            blob[r0 + NIN, XOFF:XOFF + SL] = 1.0
        blob[0:H, FLAGOFF] = float(half)
        in_maps.append({"wx": blob.astype(ml_dtypes.bfloat16)})

    results = run_bass_kernel_spmd(nc, in_maps, list(range(N_CORES))).results

    out = np.empty((B, L, H), np.float32)
    for core in range(N_CORES):
        bi, half = core // 2, core % 2
        y = np.asarray(results[core]["y"]).astype(np.float32)   # (64, 1024)
        out[bi, half * TPC:(half + 1) * TPC] = y.T
    return out


# revision 4
# speedup vs baseline: 1.0053x; 1.0053x over previous
"""Trainium2 Bass kernel for nn_CellLayer_25752623907073.

The reference is an init-guess network (MLP/S4D stack) followed by a DEER
quasi-Newton parallel solve of a GRU recurrence.  DEER is a contraction: it
converges to the sequential GRU trajectory from any initial guess, so the
init-guess network has no effect on the output and the task reduces to
evaluating the GRU trajectory.

The kernel solves the GRU by quasi-DEER fixed-point iteration with a
DIAGONAL linear solve: each round evaluates all gates in parallel at the
lagged previous iterate h~[t-1], then propagates the exact diagonal
recurrence h[t] = z[t]*h[t-1] + (1-z[t])*a[t] along the sequence with DVE
tensor_tensor_scan ops (state fp32 inside the scan).  The fixed point is
the true trajectory; measured contraction ~0.26/round, so ROUNDS=4 puts
iteration error under the bf16 floor (total rel err ~8e-3 vs 2e-2 gate).

Sharding: 8 cores = 4 batches x 2 sequence halves, no collectives.  Each
core owns a window of MARG+1024 positions (MARG=16 warm-in cols, discarded:
zero-padded+masked on first-half cores, real inputs on second-half cores
whose initial-state error decays through them).  The window is split into
two 528-col segments stacked on partitions (seg0 -> 0:64, seg1 -> 64:128),
so each ACT/DVE op covers 2x columns per instruction; matmuls use
block-diagonal stationaries.  Segment scan initials are 0 every round: the
error this injects decays below 1e-3 within the MARG warm-in cols.

Tiles are asymmetric [132, 396]: the round-to-round critical cycle runs
through tile 0 only (its scan feeds the next round's h-matmuls), so tile 0
is small to keep the chain short while tile 1 rides the engine slack.
Emission order per engine follows steady-state readiness (the completion-
counter semaphores entangle waiters when instructions park out of order).
The last round's tile-1 scan is split into 132-col chunks so the output
DMAs (bf16, converted on host) pipeline behind compute on the SP/ACT/Pool
HWDGE queues.
"""

import numpy as np
import ml_dtypes

import concourse.bacc as bacc
import concourse.mybir as mybir
import concourse.tile as tile
from concourse.bass_utils import run_bass_kernel_spmd

F32 = mybir.dt.float32
BF16 = mybir.dt.bfloat16
AF = mybir.ActivationFunctionType
ALU = mybir.AluOpType

B, L, NIN, H = 4, 2048, 32, 64
TPC = L // 2          # timesteps per core
MARG = 16             # warm-in columns per segment (discarded)
SL = MARG + 512       # segment length (528)
NW = MARG + TPC       # window length (1040)
NSEG = 2
TLS = [264, 264]      # tile column sizes (sum = SL)
FIN_CHUNK = 132       # last-round scan/DMA chunk size
ROUNDS = 4
N_CORES = 8
XR = 2 * (NIN + 1)    # x2 rows: 2 segs x (x + ones)

# blob columns (bf16, [128, BLOBCOLS]) — DMA'd in 4 pieces:
#   A: [0:514]  wxr|wxz|wxa block-diag (rows: Wg^T + bias row, per seg),
#               I128, bn2, flag           (Pool queue, needed by round 0)
#   B: x2 tile-0 cols                      (SP queue)
#   C: x2 tile-1 cols                      (SP queue)
#   D: uhr|uhz|uha block-diag              (ACT queue, needed by round 1)
WOFF = {"wxr": 0, "wxz": 128, "wxa": 256, "ident": 384}
BNOFF = 512
FLAGOFF = 513
XOFF = 514
UOFF = {"uhr": XOFF + SL, "uhz": XOFF + SL + 128, "uha": XOFF + SL + 256}
BLOBCOLS = XOFF + SL + 384


def _build_program():
    nc = bacc.Bacc("TRN2", debug=False)

    wx = nc.declare_dram_parameter("wx", [128, BLOBCOLS], BF16, isOutput=False)
    yout = nc.declare_dram_parameter("y", [H, L // 2], BF16, isOutput=True)

    with tile.TileContext(nc) as tc:
        with (
            tc.tile_pool(name="const", bufs=1) as cpool,
            tc.tile_pool(name="tmp", bufs=2) as tmp,
            tc.tile_pool(name="psum_r", bufs=2, space="PSUM") as psum_r,
            tc.tile_pool(name="psum_z", bufs=2, space="PSUM") as psum_z,
            tc.tile_pool(name="psum_a1", bufs=2, space="PSUM") as psum_a1,
            tc.tile_pool(name="psum_ia", bufs=2, space="PSUM") as psum_ia,
        ):
            t_wx = cpool.tile([128, BLOBCOLS], BF16)
            nc.sync.dma_start(t_wx[:, 0:XOFF], wx[:, 0:XOFF])
            nc.scalar.dma_start(t_wx[:, XOFF:XOFF + TLS[0]],
                                wx[:, XOFF:XOFF + TLS[0]])
            nc.sync.dma_start(t_wx[:, XOFF + TLS[0]:XOFF + SL],
                              wx[:, XOFF + TLS[0]:XOFF + SL])
            nc.scalar.dma_start(t_wx[:, XOFF + SL:BLOBCOLS],
                                wx[:, XOFF + SL:BLOBCOLS])

            wst = {k: t_wx[:, off:off + 128] for k, off in UOFF.items()}
            wst["ident"] = t_wx[:, WOFF["ident"]:WOFF["ident"] + 128]
            wx_x = {k: t_wx[0:XR, WOFF[k]:WOFF[k] + 128]
                    for k in ("wxr", "wxz", "wxa")}

            # PE p-state warm-up: dummy matmuls on a zeroed tile so the ramp
            # to full clock runs during the input DMA.
            t_zero = cpool.tile([128, 264], BF16)
            nc.vector.memset(t_zero[:], 0.0)
            for i in range(2):
                p_w = psum_ia.tile([128, 264], F32, tag="pia", name=f"warm{i}")
                nc.tensor.matmul(p_w[:], t_zero[:, 0:128], t_zero[:],
                                 start=True, stop=True)

            # warm the sigmoid/tanh ACT tables during the input DMA
            t_warm = cpool.tile([1, 2], F32)
            nc.vector.memset(t_warm[:], 0.0)
            nc.scalar.activation(t_warm[:, 0:1], t_warm[:, 0:1], AF.Sigmoid)
            nc.scalar.activation(t_warm[:, 1:2], t_warm[:, 1:2], AF.Tanh)

            # bn/flag as f32 via DVE so sweep ops never wait on the DMA sem
            t_bnflag = cpool.tile([128, 2], F32)
            nc.vector.tensor_copy(t_bnflag[:], t_wx[:, BNOFF:BNOFF + 2])
            t_bn = t_bnflag[:, 0:1]
            t_flag = t_bnflag[0:H, 1:2]

            # h~ double buffers: col 0 = 0 (state before the segment), col
            # 1+c = state at segment-local col c.
            hbuf = [cpool.tile([128, 1 + SL], BF16, name=f"h{i}")
                    for i in range(2)]
            nc.vector.memset(hbuf[0][:, 0:1], 0.0)
            nc.vector.memset(hbuf[1][:, 0:1], 0.0)

            def x2(c0, tl):
                return t_wx[0:XR, XOFF + c0:XOFF + c0 + tl]

            def emit_round(k):
                hprev = hbuf[k % 2]
                hnew = hbuf[(k + 1) % 2]
                ntile = len(TLS)
                ps = {}
                gate = {}
                # PE: all tiles' pre-act matmuls in readiness order, the
                # ident-adds (which wait on t1) after them.
                c0 = 0
                for t, tl in enumerate(TLS):
                    hp = hprev[:, c0:c0 + tl]
                    xa = x2(c0, tl)
                    p_r = psum_r.tile([128, tl], F32, tag="pr",
                                      name=f"pr_{k}_{t}")
                    p_z = psum_z.tile([128, tl], F32, tag="pz",
                                      name=f"pz_{k}_{t}")
                    p_ia = psum_ia.tile([128, tl], F32, tag="pia",
                                        name=f"pia_{k}_{t}")
                    p_a1 = None
                    if k > 0:
                        p_a1 = psum_a1.tile([128, tl], F32, tag="pa1",
                                            name=f"pa1_{k}_{t}")
                        nc.tensor.matmul(p_r[:], wst["uhr"], hp,
                                         start=True, stop=False)
                        nc.tensor.matmul(p_r[:], wx_x["wxr"], xa,
                                         start=False, stop=True)
                        nc.tensor.matmul(p_a1[:], wst["uha"], hp,
                                         start=True, stop=True,
                                         skip_group_check=True)
                        nc.tensor.matmul(p_z[:], wst["uhz"], hp,
                                         start=True, stop=False,
                                         skip_group_check=True)
                        nc.tensor.matmul(p_z[:], wx_x["wxz"], xa,
                                         start=False, stop=True,
                                         skip_group_check=True)
                    else:
                        nc.tensor.matmul(p_r[:], wx_x["wxr"], xa,
                                         start=True, stop=True)
                        nc.tensor.matmul(p_z[:], wx_x["wxz"], xa,
                                         start=True, stop=True,
                                         skip_group_check=True)
                    nc.tensor.matmul(p_ia[:], wx_x["wxa"], xa,
                                     start=True, stop=False,
                                     skip_group_check=True)
                    ps[t] = (p_r, p_z, p_a1, p_ia)
                    c0 += tl

                # ACT sigmoids for r: tile order; z(t0); then tanh(t0)
                # before z(t1)/tanh(t1) (readiness order).
                for t in range(ntile):
                    p_r, p_z, p_a1, p_ia = ps[t]
                    tl = TLS[t]
                    rt = tmp.tile([128, tl], BF16, tag="rt", name=f"rt{k}_{t}")
                    zt = tmp.tile([128, tl], BF16, tag="zt", name=f"zt{k}_{t}")
                    t1 = tmp.tile([128, tl], BF16, tag="t1", name=f"t1{k}_{t}")
                    at = tmp.tile([128, tl], BF16, tag="at", name=f"at{k}_{t}")
                    gt = tmp.tile([128, tl], BF16, tag="gt", name=f"gt{k}_{t}")
                    gate[t] = (rt, zt, t1, at, gt)

                nc.scalar.activation(gate[0][0][:], ps[0][0][:], AF.Sigmoid)
                nc.scalar.activation(gate[0][1][:], ps[0][1][:], AF.Sigmoid)

                # DVE t1 ops interleave with ACT by readiness
                def emit_t1(t):
                    rt, zt, t1, at, gt = gate[t]
                    p_r, p_z, p_a1, p_ia = ps[t]
                    if k > 0:
                        nc.vector.scalar_tensor_tensor(
                            t1[:], in0=p_a1[:], scalar=t_bn, in1=rt[:],
                            op0=ALU.add, op1=ALU.mult)
                    else:
                        nc.vector.tensor_scalar_mul(t1[:], rt[:], t_bn)
                    nc.tensor.matmul(p_ia[:], wst["ident"], t1[:],
                                     start=False, stop=True,
                                     skip_group_check=True)

                emit_t1(0)
                nc.scalar.activation(gate[1][0][:], ps[1][0][:], AF.Sigmoid)
                nc.scalar.activation(gate[0][3][:], ps[0][3][:], AF.Tanh)
                emit_t1(1)
                nc.scalar.activation(gate[1][1][:], ps[1][1][:], AF.Sigmoid)
                nc.scalar.activation(gate[1][3][:], ps[1][3][:], AF.Tanh)

                # DVE: g/mask/scan per tile in readiness order; the last
                # round's tile-1 scan is chunked so output DMAs pipeline.
                dmaq = [nc.sync, nc.scalar]
                dmai = [0]

                def emit_dma(blo, bhi):
                    ylo, yhi = blo - 1 - MARG, bhi - 1 - MARG
                    q0 = dmaq[dmai[0] % 2]
                    q1 = dmaq[(dmai[0] + 1) % 2]
                    dmai[0] += 1
                    q0.dma_start(yout[:, ylo:yhi], hnew[0:H, blo:bhi])
                    q1.dma_start(yout[:, 512 + ylo:512 + yhi],
                                 hnew[H:128, blo:bhi])

                rt0, zt0, t10, at0, gt0 = gate[0]
                nc.vector.scalar_tensor_tensor(
                    gt0[:], in0=zt0[:], scalar=1.0, in1=at0[:],
                    op0=ALU.subtract, op1=ALU.mult)
                nc.vector.tensor_scalar_mul(
                    gt0[0:H, 0:MARG], gt0[0:H, 0:MARG], t_flag)
                nc.vector.tensor_tensor_scan(
                    hnew[:, 1:1 + TLS[0]], zt0[:], gt0[:], 0.0,
                    ALU.mult, ALU.subtract)
                if k == ROUNDS - 1:
                    emit_dma(1 + MARG, 1 + TLS[0])

                rt1, zt1, t11, at1, gt1 = gate[1]
                nc.vector.scalar_tensor_tensor(
                    gt1[:], in0=zt1[:], scalar=1.0, in1=at1[:],
                    op0=ALU.subtract, op1=ALU.mult)
                if k < ROUNDS - 1:
                    nc.vector.tensor_tensor_scan(
                        hnew[:, 1 + TLS[0]:1 + SL], zt1[:], gt1[:],
                        hnew[:, TLS[0]:TLS[0] + 1], ALU.mult, ALU.subtract)
                else:
                    c0 = TLS[0]
                    while c0 < SL:
                        ce = min(c0 + FIN_CHUNK, SL)
                        nc.vector.tensor_tensor_scan(
                            hnew[:, 1 + c0:1 + ce],
                            zt1[:, c0 - TLS[0]:ce - TLS[0]],
                            gt1[:, c0 - TLS[0]:ce - TLS[0]],
                            hnew[:, c0:c0 + 1], ALU.mult, ALU.subtract)
                        emit_dma(1 + c0, 1 + ce)
                        c0 = ce

            for k in range(ROUNDS):
                emit_round(k)

    nc.compile()
    return nc


_CACHE = {}


def kernel(**inputs):
    xs = np.asarray(inputs["xs"], np.float32)
    w_ih = np.asarray(inputs["w_ih"], np.float32)
    w_hh = np.asarray(inputs["w_hh"], np.float32)
    b_gru = np.asarray(inputs["b_gru"], np.float32)
    bn_gru = np.asarray(inputs["bn_gru"], np.float32)

    if "nc" not in _CACHE:
        _CACHE["nc"] = _build_program()
    nc = _CACHE["nc"]

    base = np.zeros((128, BLOBCOLS), np.float32)
    for gi, key in enumerate(("wxr", "wxz", "wxa")):
        wg = w_ih[gi * H:(gi + 1) * H]          # (H, NIN)
        bg = b_gru[gi * H:(gi + 1) * H]
        for s in range(NSEG):
            r0 = s * (NIN + 1)
            cblk = WOFF[key] + s * H
            base[r0:r0 + NIN, cblk:cblk + H] = wg.T
            base[r0 + NIN, cblk:cblk + H] = bg
    for gi, key in enumerate(("uhr", "uhz", "uha")):
        ug = w_hh[gi * H:(gi + 1) * H]          # (H, H)
        for s in range(NSEG):
            base[s * H:(s + 1) * H,
                 UOFF[key] + s * H:UOFF[key] + (s + 1) * H] = ug.T
    base[:, WOFF["ident"]:WOFF["ident"] + 128] = np.eye(128, dtype=np.float32)
    base[0:H, BNOFF] = bn_gru
    base[H:128, BNOFF] = bn_gru

    in_maps = []
    for core in range(N_CORES):
        bi, half = core // 2, core % 2
        p0 = half * TPC - MARG
        xw = np.zeros((NW, NIN), np.float32)
        lo = max(0, p0)
        xw[lo - p0:] = xs[bi, lo:p0 + NW]
        blob = base.copy()
        for s in range(NSEG):
            r0 = s * (NIN + 1)
            xsg = xw[512 * s:512 * s + SL]       # (SL, NIN)
            blob[r0:r0 + NIN, XOFF:XOFF + SL] = xsg.T
            blob[r0 + NIN, XOFF:XOFF + SL] = 1.0
        blob[0:H, FLAGOFF] = float(half)
        in_maps.append({"wx": blob.astype(ml_dtypes.bfloat16)})

    results = run_bass_kernel_spmd(nc, in_maps, list(range(N_CORES))).results

    out = np.empty((B, L, H), np.float32)
    for core in range(N_CORES):
        bi, half = core // 2, core % 2
        y = np.asarray(results[core]["y"]).astype(np.float32)   # (64, 1024)
        out[bi, half * TPC:(half + 1) * TPC] = y.T
    return out
